# revision 15
# baseline (speedup 1.0000x reference)
"""Trainium2 Bass kernel for a dense pre-LN transformer block.

Shapes (hardcoded): B=2, S=2048, D=1024, H=16, HD=64, F=4096, fp32 I/O.

Sharding: token-parallel over 8 cores. Flatten (B,S) -> 4096 tokens; core i
owns 512 tokens (batch b = i//4, quarter j = i%4). Attention needs K/V for
the full 2048-token sequence of the core's batch, so each core recomputes
LN1 + K/V projections for all 2048 batch tokens (no collectives needed).
Each core's xTb input has its own 512 tokens rotated to the front, so the
same SPMD program works on every core (attention is permutation-invariant
over the key/value axis).

Layout: "transposed" activations throughout - features on SBUF partitions,
tokens on the free dim. Chained matmuls stay natural (host pre-transposes
the weights, which act as the stationary operand) and per-feature biases /
LN gains are per-partition [P,1] operands. LayerNorm reductions (over
features = partitions) use ones-vector matmuls; per-token stats broadcast
back across partitions with a K=1 ones matmul.

Numerics: matmul operands bf16 (PSUM accumulation fp32); residual stream
and attention accumulators fp32. Softmax skips max-subtraction (logits are
~N(0, 0.41^2); exp cannot overflow). The softmax division is deferred: a
ones-column in the augmented V computes per-(head,token) denominators in
the same matmuls that compute attn @ V (output row 64 of each 65-row AV
accumulation).
"""

import numpy as np
import ml_dtypes

P = 128
D = 1024
F = 4096
H = 16
HD = 64
SC = 512          # tokens per core (own)
T = 2048          # tokens per batch (attention span)
NCHUNK = 4        # T / SC
DK = D // P       # 8 feature tiles
FK = F // P       # 32 hidden tiles
HE_AUG = H * (HD + 1)   # v columns: per head 64 v-dims + 1 ones col (1040)
EPS = 1e-5

_CACHE = {}


def _build_nc():
    import concourse.bass as bass
    import concourse.mybir as mybir
    import concourse.tile as tile
    from concourse.bass import ts
    from contextlib import ExitStack

    dt = mybir.dt
    f32 = dt.float32
    bf16 = dt.bfloat16
    AF = mybir.ActivationFunctionType
    OP = mybir.AluOpType

    from concourse import bacc

    nc = bacc.Bacc()

    # ---- DRAM I/O ----
    xT = nc.dram_tensor("xT", [D, SC], f32, kind="ExternalInput")
    xTb = nc.dram_tensor("xTb", [D, T], bf16, kind="ExternalInput")
    WqT = nc.dram_tensor("WqT", [D, D], bf16, kind="ExternalInput")
    WkT = nc.dram_tensor("WkT", [D, D], bf16, kind="ExternalInput")
    WvaT = nc.dram_tensor("WvaT", [D, HE_AUG], bf16, kind="ExternalInput")
    WoTt = nc.dram_tensor("WoTt", [DK, H, HD, P], bf16, kind="ExternalInput")
    W1Tt = nc.dram_tensor("W1Tt", [FK, D, P], bf16, kind="ExternalInput")
    W2Tt = nc.dram_tensor("W2Tt", [DK, F, P], bf16, kind="ExternalInput")
    bqs = nc.dram_tensor("bqs", [D], f32, kind="ExternalInput")
    bk = nc.dram_tensor("bk", [D], f32, kind="ExternalInput")
    bva = nc.dram_tensor("bva", [HE_AUG], f32, kind="ExternalInput")
    bo = nc.dram_tensor("bo", [D], f32, kind="ExternalInput")
    b1 = nc.dram_tensor("b1", [F], f32, kind="ExternalInput")
    b2 = nc.dram_tensor("b2", [D], f32, kind="ExternalInput")
    g1 = nc.dram_tensor("g1", [D], f32, kind="ExternalInput")
    c1 = nc.dram_tensor("c1", [D], f32, kind="ExternalInput")
    g2 = nc.dram_tensor("g2", [D], f32, kind="ExternalInput")
    c2 = nc.dram_tensor("c2", [D], f32, kind="ExternalInput")
    out = nc.dram_tensor("outT", [D, SC], f32, kind="ExternalOutput")

    pp = lambda a: a.rearrange("(m p) -> p m", p=P)
    kp3 = lambda a: a.rearrange("(k p) n -> p k n", p=P)

    def ln_stats(tc_pools, xb, sq_pool, s_pool, ps_pool, ones_k, eps_t,
                 mu_out, rstd_out):
        """xb: [P, DK, SC] bf16 -> write per-token mu/rstd (bf16) slices."""
        sq = sq_pool.tile([P, DK, SC], bf16, tag="sq")
        for k in range(DK):
            nc.vector.tensor_tensor(out=sq[:, k, :], in0=xb[:, k, :],
                                    in1=xb[:, k, :], op=OP.mult)
        ps_sum = ps_pool.tile([1, SC], f32, tag="st")
        ps_ssq = ps_pool.tile([1, SC], f32, tag="st")
        for k in range(DK):
            nc.tensor.matmul(ps_sum, lhsT=ones_k, rhs=xb[:, k, :],
                             start=(k == 0), stop=(k == DK - 1))
        for k in range(DK):
            nc.tensor.matmul(ps_ssq, lhsT=ones_k, rhs=sq[:, k, :],
                             start=(k == 0), stop=(k == DK - 1))
        mu = s_pool.tile([1, SC], f32, tag="mu")
        nc.vector.tensor_scalar_mul(mu, ps_sum, 1.0 / D)
        ss = s_pool.tile([1, SC], f32, tag="ss")
        nc.vector.tensor_scalar_mul(ss, ps_ssq, 1.0 / D)
        var = s_pool.tile([1, SC], f32, tag="var")
        nc.vector.tensor_tensor(out=var, in0=mu, in1=mu, op=OP.mult)
        nc.vector.tensor_tensor(out=var, in0=ss, in1=var, op=OP.subtract)
        sd = s_pool.tile([1, SC], f32, tag="sd")
        nc.scalar.activation(out=sd, in_=var, func=AF.Sqrt, bias=eps_t)
        rstd = s_pool.tile([1, SC], f32, tag="rstd")
        nc.vector.reciprocal(out=rstd, in_=sd)
        nc.vector.tensor_copy(out=mu_out, in_=mu)
        nc.vector.tensor_copy(out=rstd_out, in_=rstd)

    with tile.TileContext(nc) as tc, ExitStack() as top:
        singles = top.enter_context(tc.tile_pool(name="singles", bufs=1))

        ones_k = singles.tile([P, 1], bf16)
        nc.vector.memset(ones_k, 1.0)
        ones_m = singles.tile([1, P], bf16)
        nc.vector.memset(ones_m, 1.0)
        eps_t = singles.tile([1, 1], f32)
        nc.vector.memset(eps_t, EPS)

        bq_sb = singles.tile([P, DK], f32)
        nc.gpsimd.dma_start(out=bq_sb, in_=pp(bqs[:]))
        bk_sb = singles.tile([P, DK], f32)
        nc.gpsimd.dma_start(out=bk_sb, in_=pp(bk[:]))
        bo_sb = singles.tile([P, DK], f32)
        nc.gpsimd.dma_start(out=bo_sb, in_=pp(bo[:]))
        b2_sb = singles.tile([P, DK], f32)
        nc.gpsimd.dma_start(out=b2_sb, in_=pp(b2[:]))
        b1_sb = singles.tile([P, FK], f32)
        nc.gpsimd.dma_start(out=b1_sb, in_=pp(b1[:]))
        g1_sb = singles.tile([P, DK], f32)
        nc.gpsimd.dma_start(out=g1_sb, in_=pp(g1[:]))
        c1_sb = singles.tile([P, DK], f32)
        nc.gpsimd.dma_start(out=c1_sb, in_=pp(c1[:]))
        g2_sb = singles.tile([P, DK], f32)
        nc.gpsimd.dma_start(out=g2_sb, in_=pp(g2[:]))
        c2_sb = singles.tile([P, DK], f32)
        nc.gpsimd.dma_start(out=c2_sb, in_=pp(c2[:]))

        bva_bc = singles.tile([P, HE_AUG], f32)
        bva_src = bass.AP(tensor=bva[:].tensor, offset=bva[:].offset,
                          ap=[[0, P]] + list(bva[:].ap))
        nc.gpsimd.dma_start(out=bva_bc, in_=bva_src)

        WkT_sb = singles.tile([P, DK, D], bf16)
        for k in range(DK):
            nc.sync.dma_start(out=WkT_sb[:, k, :], in_=WkT[ts(k, P), :])
        WvaT_sb = singles.tile([P, DK, HE_AUG], bf16)
        for k in range(DK):
            nc.sync.dma_start(out=WvaT_sb[:, k, :], in_=WvaT[ts(k, P), :])

        mu1_all = singles.tile([1, T], bf16)
        rstd1_all = singles.tile([1, T], bf16)

        # ---------- phase 1: LN1 statistics over the full batch ----------
        with tc.tile_pool(name="st_x", bufs=4) as st_x, \
             tc.tile_pool(name="st_t", bufs=2) as st_t, \
             tc.tile_pool(name="st_s", bufs=2) as st_s, \
             tc.tile_pool(name="st_ps", bufs=4, space="PSUM") as st_ps:
            for c in range(NCHUNK):
                xb = st_x.tile([P, DK, SC], bf16, tag="xb")
                for k in range(DK):
                    nc.gpsimd.dma_start(out=xb[:, k, :],
                                        in_=xTb[ts(k, P), ts(c, SC)])
                ln_stats(None, xb, st_t, st_s, st_ps, ones_k, eps_t,
                         mu1_all[:, ts(c, SC)], rstd1_all[:, ts(c, SC)])

        # ---------- phase 2: chunk loop ----------
        res_x = top.enter_context(tc.tile_pool(name="res_x", bufs=1))
        xt_own = res_x.tile([P, DK, SC], f32)
        for k in range(DK):
            nc.sync.dma_start(out=xt_own[:, k, :], in_=xT[ts(k, P), :])

        acc_p = top.enter_context(tc.tile_pool(name="acc", bufs=1))
        acc65 = acc_p.tile([HD + 1, H, SC], f32)

        with tc.tile_pool(name="qT", bufs=1) as qT_p, \
             tc.tile_pool(name="c_x", bufs=2) as c_x, \
             tc.tile_pool(name="c_h1", bufs=2) as c_h1, \
             tc.tile_pool(name="c_kt", bufs=2) as c_kt, \
             tc.tile_pool(name="c_vt", bufs=2) as c_vt, \
             tc.tile_pool(name="c_pt", bufs=2) as c_pt, \
             tc.tile_pool(name="c_tmp", bufs=2) as c_tmp, \
             tc.tile_pool(name="c_bc", bufs=2) as c_bc, \
             tc.tile_pool(name="psA", bufs=4, space="PSUM") as psA, \
             tc.tile_pool(name="psB", bufs=2, space="PSUM") as psB, \
             ExitStack() as wq_ctx:
            wq_pool = wq_ctx.enter_context(tc.tile_pool(name="wq", bufs=1))
            WqT_sb = wq_pool.tile([P, DK, D], bf16)
            for k in range(DK):
                nc.sync.dma_start(out=WqT_sb[:, k, :], in_=WqT[ts(k, P), :])
            qt = qT_p.tile([P, DK, SC], bf16)

            for c in range(NCHUNK):
                xb = c_x.tile([P, DK, SC], bf16, tag="xb")
                for k in range(DK):
                    nc.gpsimd.dma_start(out=xb[:, k, :],
                                        in_=xTb[ts(k, P), ts(c, SC)])

                mub_ps = psA.tile([P, SC], f32, tag="ps")
                nc.tensor.matmul(mub_ps, lhsT=ones_m, rhs=mu1_all[:, ts(c, SC)],
                                 start=True, stop=True)
                rsb_ps = psA.tile([P, SC], f32, tag="ps")
                nc.tensor.matmul(rsb_ps, lhsT=ones_m, rhs=rstd1_all[:, ts(c, SC)],
                                 start=True, stop=True)
                mu_bc = c_bc.tile([P, SC], bf16, tag="mu_bc")
                nc.vector.tensor_copy(out=mu_bc, in_=mub_ps)
                rstd_bc = c_bc.tile([P, SC], bf16, tag="rstd_bc")
                nc.vector.tensor_copy(out=rstd_bc, in_=rsb_ps)

                h1 = c_h1.tile([P, DK, SC], bf16, tag="h1")
                for k in range(DK):
                    t1 = c_tmp.tile([P, SC], f32, tag="t1")
                    nc.vector.tensor_tensor(out=t1, in0=xb[:, k, :], in1=mu_bc,
                                            op=OP.subtract)
                    nc.vector.tensor_tensor(out=t1, in0=t1, in1=rstd_bc, op=OP.mult)
                    nc.vector.tensor_scalar(out=h1[:, k, :], in0=t1,
                                            scalar1=g1_sb[:, k:k + 1],
                                            scalar2=c1_sb[:, k:k + 1],
                                            op0=OP.mult, op1=OP.add)

                kt = c_kt.tile([P, DK, SC], bf16, tag="kt")
                for m in range(DK):
                    ps = psA.tile([P, SC], f32, tag="ps")
                    for k in range(DK):
                        nc.tensor.matmul(ps, lhsT=WkT_sb[:, k, ts(m, P)],
                                         rhs=h1[:, k, :],
                                         start=(k == 0), stop=(k == DK - 1))
                    nc.vector.tensor_scalar(out=kt[:, m, :], in0=ps,
                                            scalar1=bk_sb[:, m:m + 1], scalar2=None,
                                            op0=OP.add)

                vt = c_vt.tile([P, NCHUNK, HE_AUG], bf16, tag="vt")
                for tm in range(NCHUNK):
                    for n0, nsz in ((0, 512), (512, 512), (1024, 16)):
                        ps = psA.tile([P, SC], f32, tag="ps")
                        for k in range(DK):
                            nc.tensor.matmul(ps[:, :nsz],
                                             lhsT=h1[:, k, ts(tm, P)],
                                             rhs=WvaT_sb[:, k, n0:n0 + nsz],
                                             start=(k == 0), stop=(k == DK - 1))
                        nc.vector.tensor_tensor(out=vt[:, tm, n0:n0 + nsz],
                                                in0=ps[:, :nsz],
                                                in1=bva_bc[:, n0:n0 + nsz],
                                                op=OP.add)

                if c == 0:
                    for m in range(DK):
                        ps = psA.tile([P, SC], f32, tag="ps")
                        for k in range(DK):
                            nc.tensor.matmul(ps, lhsT=WqT_sb[:, k, ts(m, P)],
                                             rhs=h1[:, k, :],
                                             start=(k == 0), stop=(k == DK - 1))
                        nc.vector.tensor_scalar(out=qt[:, m, :], in0=ps,
                                                scalar1=0.125,
                                                scalar2=bq_sb[:, m:m + 1],
                                                op0=OP.mult, op1=OP.add)
                    wq_ctx.close()

                for hp in range(DK):
                    h0, h1h = 2 * hp, 2 * hp + 1
                    av0 = psB.tile([HD + 1, SC], f32, tag="av0")
                    av1 = psB.tile([HD + 1, SC], f32, tag="av1")
                    for tt in range(NCHUNK):
                        s0 = psA.tile([P, SC], f32, tag="ps")
                        nc.tensor.matmul(s0, lhsT=kt[0:HD, hp, ts(tt, P)],
                                         rhs=qt[0:HD, hp, :], start=True, stop=True)
                        s1 = psA.tile([P, SC], f32, tag="ps")
                        nc.tensor.matmul(s1, lhsT=kt[HD:P, hp, ts(tt, P)],
                                         rhs=qt[HD:P, hp, :], start=True, stop=True)
                        p0 = c_pt.tile([P, SC], bf16, tag="p0")
                        nc.scalar.activation(out=p0, in_=s0, func=AF.Exp)
                        p1 = c_pt.tile([P, SC], bf16, tag="p1")
                        nc.scalar.activation(out=p1, in_=s1, func=AF.Exp)
                        nc.tensor.matmul(av0, lhsT=vt[:, tt, h0 * 65:(h0 + 1) * 65],
                                         rhs=p0, start=(tt == 0),
                                         stop=(tt == NCHUNK - 1))
                        nc.tensor.matmul(av1, lhsT=vt[:, tt, h1h * 65:(h1h + 1) * 65],
                                         rhs=p1, start=(tt == 0),
                                         stop=(tt == NCHUNK - 1))
                    if c == 0:
                        nc.vector.tensor_copy(out=acc65[:, h0, :], in_=av0)
                        nc.vector.tensor_copy(out=acc65[:, h1h, :], in_=av1)
                    else:
                        nc.vector.tensor_tensor(out=acc65[:, h0, :],
                                                in0=acc65[:, h0, :], in1=av0,
                                                op=OP.add)
                        nc.vector.tensor_tensor(out=acc65[:, h1h, :],
                                                in0=acc65[:, h1h, :], in1=av1,
                                                op=OP.add)

        # ---------- phase 3: normalize, output proj, residual, LN2 ----------
        with tc.tile_pool(name="p3t", bufs=2) as p3t, \
             tc.tile_pool(name="p3s", bufs=1) as p3s, \
             tc.tile_pool(name="x2p", bufs=1) as x2p, \
             tc.tile_pool(name="h2p", bufs=1) as h2p, \
             tc.tile_pool(name="psA2", bufs=4, space="PSUM") as psA2, \
             tc.tile_pool(name="psB2", bufs=2, space="PSUM") as psB2:

            x2T = x2p.tile([P, DK, SC], f32)
            h2 = h2p.tile([P, DK, SC], bf16)

            with tc.tile_pool(name="attnS_p", bufs=1) as attnS_p, \
                 tc.tile_pool(name="wo_s", bufs=2) as wo_s:
                attnS = attnS_p.tile([HD, H, SC], bf16)
                for h in range(H):
                    rs32 = p3s.tile([1, SC], f32, tag="rs32")
                    nc.vector.reciprocal(out=rs32, in_=acc65[HD:HD + 1, h, :])
                    rs = p3s.tile([1, SC], bf16, tag="rs")
                    nc.vector.tensor_copy(out=rs, in_=rs32)
                    rb = psB2.tile([HD, SC], f32, tag="sb")
                    nc.tensor.matmul(rb, lhsT=ones_m[:, 0:HD], rhs=rs,
                                     start=True, stop=True)
                    nc.vector.tensor_tensor(out=attnS[:, h, :],
                                            in0=acc65[0:HD, h, :],
                                            in1=rb, op=OP.mult)

                for m in range(DK):
                    wot = wo_s.tile([HD, H, P], bf16, tag="wot")
                    for h in range(H):
                        nc.sync.dma_start(out=wot[:, h, :], in_=WoTt[m, h])
                    ps = psA2.tile([P, SC], f32, tag="ps")
                    for h in range(H):
                        nc.tensor.matmul(ps, lhsT=wot[:, h, :],
                                         rhs=attnS[:, h, :],
                                         start=(h == 0), stop=(h == H - 1))
                    t = p3t.tile([P, SC], f32, tag="t")
                    nc.vector.tensor_scalar(out=t, in0=ps,
                                            scalar1=bo_sb[:, m:m + 1],
                                            scalar2=None, op0=OP.add)
                    nc.vector.tensor_tensor(out=x2T[:, m, :], in0=t,
                                            in1=xt_own[:, m, :], op=OP.add)

            # LN2
            mu2 = p3s.tile([1, SC], bf16, tag="mu2")
            rstd2 = p3s.tile([1, SC], bf16, tag="rstd2")
            with tc.tile_pool(name="ln2_t", bufs=1) as ln2_t:
                xb2 = ln2_t.tile([P, DK, SC], bf16, tag="xb2")
                for k in range(DK):
                    nc.vector.tensor_copy(out=xb2[:, k, :], in_=x2T[:, k, :])
                ln_stats(None, xb2, ln2_t, p3s, psB2, ones_k, eps_t, mu2, rstd2)

            mub_ps = psA2.tile([P, SC], f32, tag="ps")
            nc.tensor.matmul(mub_ps, lhsT=ones_m, rhs=mu2, start=True, stop=True)
            rsb_ps = psA2.tile([P, SC], f32, tag="ps")
            nc.tensor.matmul(rsb_ps, lhsT=ones_m, rhs=rstd2, start=True, stop=True)
            mu_bc = p3t.tile([P, SC], f32, tag="mu_bc2")
            nc.vector.tensor_copy(out=mu_bc, in_=mub_ps)
            rstd_bc = p3t.tile([P, SC], f32, tag="rstd_bc2")
            nc.vector.tensor_copy(out=rstd_bc, in_=rsb_ps)

            for k in range(DK):
                t1 = p3t.tile([P, SC], f32, tag="t1")
                nc.vector.tensor_tensor(out=t1, in0=x2T[:, k, :], in1=mu_bc,
                                        op=OP.subtract)
                nc.vector.tensor_tensor(out=t1, in0=t1, in1=rstd_bc, op=OP.mult)
                nc.vector.tensor_scalar(out=h2[:, k, :], in0=t1,
                                        scalar1=g2_sb[:, k:k + 1],
                                        scalar2=c2_sb[:, k:k + 1],
                                        op0=OP.mult, op1=OP.add)

            # ---------- phase 4: MLP ----------
            out3 = kp3(out[:])
            with tc.tile_pool(name="gT", bufs=1) as gT_p:
                gT = gT_p.tile([P, FK, SC], bf16)
                with tc.tile_pool(name="w1s", bufs=3) as w1s:
                    for fm in range(FK):
                        w1t = w1s.tile([P, DK, P], bf16, tag="w1t")
                        for k in range(DK):
                            nc.sync.dma_start(out=w1t[:, k, :],
                                              in_=W1Tt[fm, ts(k, P), :])
                        ps = psA2.tile([P, SC], f32, tag="ps")
                        for k in range(DK):
                            nc.tensor.matmul(ps, lhsT=w1t[:, k, :], rhs=h2[:, k, :],
                                             start=(k == 0), stop=(k == DK - 1))
                        nc.scalar.activation(out=gT[:, fm, :], in_=ps,
                                             func=AF.Gelu_apprx_tanh,
                                             bias=b1_sb[:, fm:fm + 1])

                with tc.tile_pool(name="w2s", bufs=2) as w2s:
                    for m in range(DK):
                        w2t = w2s.tile([P, FK, P], bf16, tag="w2t")
                        for k in range(FK):
                            nc.sync.dma_start(out=w2t[:, k, :],
                                              in_=W2Tt[m, ts(k, P), :])
                        ps = psA2.tile([P, SC], f32, tag="ps")
                        for k in range(FK):
                            nc.tensor.matmul(ps, lhsT=w2t[:, k, :], rhs=gT[:, k, :],
                                             start=(k == 0), stop=(k == FK - 1))
                        t = p3t.tile([P, SC], f32, tag="t")
                        nc.vector.tensor_scalar(out=t, in0=ps,
                                                scalar1=b2_sb[:, m:m + 1],
                                                scalar2=None, op0=OP.add)
                        to = p3t.tile([P, SC], f32, tag="to")
                        nc.vector.tensor_tensor(out=to, in0=t,
                                                in1=x2T[:, m, :], op=OP.add)
                        nc.sync.dma_start(out=out3[:, m, :], in_=to)

    nc.finalize()
    return nc


def _prep_inputs(inputs):
    bf16 = ml_dtypes.bfloat16
    x = np.asarray(inputs["x"], np.float32)
    Wq = np.asarray(inputs["Wq"], np.float32).reshape(D, D)
    Wk = np.asarray(inputs["Wk"], np.float32).reshape(D, D)
    Wv = np.asarray(inputs["Wv"], np.float32).reshape(D, D)
    Wo = np.asarray(inputs["Wo"], np.float32)
    W1 = np.asarray(inputs["W1"], np.float32)
    W2 = np.asarray(inputs["W2"], np.float32)

    com = {}
    com["WqT"] = np.ascontiguousarray(Wq.T).astype(bf16)
    com["WkT"] = np.ascontiguousarray(Wk.T).astype(bf16)
    WvaT = np.zeros((D, HE_AUG), np.float32)
    for h in range(H):
        WvaT[:, h * 65:h * 65 + 64] = Wv.T[:, h * 64:(h + 1) * 64]
    com["WvaT"] = WvaT.astype(bf16)
    # WoTt[m, e, h, :] = Wo[m*128:(m+1)*128, h*64+e]  (dout tiles of Wo columns)
    com["WoTt"] = np.ascontiguousarray(
        Wo.reshape(DK, P, H, HD).transpose(0, 2, 3, 1)).astype(bf16)
    W1T = np.ascontiguousarray(W1.T)
    com["W1Tt"] = np.ascontiguousarray(
        W1T.reshape(D, FK, P).transpose(1, 0, 2)).astype(bf16)
    W2T = np.ascontiguousarray(W2.T)
    com["W2Tt"] = np.ascontiguousarray(
        W2T.reshape(F, DK, P).transpose(1, 0, 2)).astype(bf16)
    com["bqs"] = (np.asarray(inputs["bq"], np.float32).reshape(D) * 0.125)
    com["bk"] = np.asarray(inputs["bk"], np.float32).reshape(D)
    bva = np.zeros(HE_AUG, np.float32)
    bvf = np.asarray(inputs["bv"], np.float32).reshape(D)
    for h in range(H):
        bva[h * 65:h * 65 + 64] = bvf[h * 64:(h + 1) * 64]
        bva[h * 65 + 64] = 1.0
    com["bva"] = bva
    com["bo"] = np.asarray(inputs["bo"], np.float32)
    com["b1"] = np.asarray(inputs["b1"], np.float32)
    com["b2"] = np.asarray(inputs["b2"], np.float32)
    com["g1"] = np.asarray(inputs["ln1_g"], np.float32)
    com["c1"] = np.asarray(inputs["ln1_b"], np.float32)
    com["g2"] = np.asarray(inputs["ln2_g"], np.float32)
    com["c2"] = np.asarray(inputs["ln2_b"], np.float32)

    in_maps = []
    for core in range(8):
        b, j = core // 4, core % 4
        xTb_full = np.ascontiguousarray(x[b].T)
        own = xTb_full[:, j * SC:(j + 1) * SC]
        rest = np.concatenate(
            [xTb_full[:, :j * SC], xTb_full[:, (j + 1) * SC:]], axis=1)
        rot = np.concatenate([own, rest], axis=1)
        m = dict(com)
        m["xT"] = np.ascontiguousarray(own).astype(np.float32)
        m["xTb"] = rot.astype(bf16)
        in_maps.append(m)
    return in_maps


def kernel(**inputs):
    from concourse.bass_utils import run_bass_kernel_spmd

    if "nc" not in _CACHE:
        _CACHE["nc"] = _build_nc()
    nc = _CACHE["nc"]

    in_maps = _prep_inputs(inputs)
    res = run_bass_kernel_spmd(nc, in_maps, core_ids=list(range(8)))

    out = np.empty((2, T, D), np.float32)
    for core in range(8):
        b, j = core // 4, core % 4
        outT = np.asarray(res.results[core]["outT"])
        out[b, j * SC:(j + 1) * SC, :] = outT.T
    return out


if __name__ == "__main__":
    nc = _build_nc()
    print("built ok, instructions:",
          sum(1 for _ in nc.m.functions[0].instructions)
          if hasattr(nc.m.functions[0], "instructions") else "n/a")
